# revision 32
# baseline (speedup 1.0000x reference)
"""
MoE-routing kernel for Trainium2 (8 NeuronCores, SPMD via bass).

Computation (matches the reference):
  attended[b, c] = sum_hw(mn[b, hw] * feat[b, c, hw]),  mn = (m+1e-10)/sum(m+1e-10)
  out[b, a]      = attended[b, :] @ W[inst[b], a, :] + bias[inst[b], a]

Strategy: channel-sharded over 8 cores (CS = 2048/8 = 256 channels each);
host sums the 8 partial [B, A] outputs and adds the bias.  Samples are
sorted by expert on the host so each expert's samples form a contiguous
range.  The mask multiply is folded into the host-side fp16 cast of feat
(fm = feat * mn), so on device the pooling is a pure free-axis sum that
runs on the DVE — the PE does only the expert GEMM.

The kernel is HBM-bandwidth bound: ~52 MB/core (fm 25.7 + W 24.6 + out
1.5 MB) against a measured ~353 GB/s per-core ceiling (shared across all
queues; SWDGE participation degrades it, so everything rides the two
HWDGE queues).  Per expert group (<=128 samples of one expert), emitted
largest-first and software-pipelined with a one-group skew:
  load stage:  per-kt fm tiles [128c, gsz, 196hw] stream on the SP
     HWDGE queue; DVE tensor_reduce sums hw directly into fp16
     att16 [128, KT, gsz] (internal fp32 accumulate); W[e] [128, KT, A]
     is dispatched on the Act HWDGE queue BEFORE the previous group's
     evictions so the weight stream never waits on PE-dependent work.
  gemm stage (one group behind): per 512-answer chunk, KT fp16 matmuls
     accumulate in PSUM; Act evicts to an SBUF row tile.
  out stage (two groups behind): the row tile DMAs to DRAM on an HWDGE
     queue, lagged so its eviction-dependent wait never head-of-line
     blocks fm/W dispatch in the engine FIFOs.
"""

import sys

if "/opt/trn_rl_repo" not in sys.path:
    sys.path.insert(0, "/opt/trn_rl_repo")

import numpy as np

import concourse.bass as bass
import concourse.mybir as mybir
import concourse.tile as tile
from concourse import bacc
from concourse import bass_utils

# Problem constants (hardcoded; kernel.py must be self-contained)
B = 256          # batch
C = 2048         # channels
HW = 196         # spatial positions (14*14)
E = 16           # experts
A = 3000         # answers
NCORES = 8
CS = C // NCORES  # channel shard per core = 256
P = 128
KT = CS // P      # channel k-tiles per core = 2
CHUNKS = [(c0, min(512, A - c0)) for c0 in range(0, A, 512)]
# W sub-tiles per group: matmuls on early answers start while later answers
# still stream, shortening the end-of-iteration tail.
WSPLITS = [(0, 1024), (1024, 1024), (2048, 952)]

F32 = mybir.dt.float32
F16 = mybir.dt.float16
AXIS_X = mybir.AxisListType.X
ADD = mybir.AluOpType.add


def _make_groups(counts):
    """[(gstart_in_sorted_order, gsz, expert)] with gsz <= 128."""
    groups = []
    start = 0
    for e in range(E):
        n = int(counts[e])
        g0 = start
        while n > 0:
            gsz = min(n, P)
            groups.append((g0, gsz, e))
            g0 += gsz
            n -= gsz
        start += int(counts[e])
    return groups


def build_program(groups, loop_n=1, do_reduce=True, do_mm=True, do_out=True,
                  fm_bufs=4, wsplit=False, reorder=True, fm_fused=False,
                  all_sync=False, tail_split=False):
    """Build + compile the per-core Bass program (identical on all cores).

    do_reduce/do_mm/do_out strip compute stages for sim ablations; the
    correctness path always uses the defaults."""
    nc = bacc.Bacc("TRN2", target_bir_lowering=False, debug=False,
                   num_devices=NCORES)

    fm_d = nc.dram_tensor("fm", [KT, P, B, HW], F16, kind="ExternalInput").ap()
    wt_d = nc.dram_tensor("wt", [E, P, KT, A], F16, kind="ExternalInput").ap()
    part_d = nc.dram_tensor("part", [B, A], F16, kind="ExternalOutput").ap()

    import contextlib
    with tile.TileContext(nc) as tc:
        loop_ctx = tc.For_i(0, loop_n, 1) if loop_n > 1 else contextlib.nullcontext()
        with (
            loop_ctx,
            tc.tile_pool(name="persist", bufs=1) as pp,
            tc.tile_pool(name="fm", bufs=fm_bufs) as fp,
            tc.tile_pool(name="wt", bufs=3) as wtp,
            tc.tile_pool(name="outs", bufs=2) as op,
            tc.tile_pool(name="ps_mm", bufs=4, space="PSUM") as pmm,
        ):
            # emit largest groups first so the end-of-iteration tail chain
            # runs on the smallest group
            order = (sorted(range(len(groups)), key=lambda i: -groups[i][1])
                     if reorder else list(range(len(groups))))
            n = len(order)
            atts, wts = {}, {}

            def load_stage(oi):
                gi = order[oi]
                g0, gsz, e = groups[gi]
                # pooling: fm -> att16 (per-kt DMA + DVE reduce)
                att16 = pp.tile([P, KT, gsz], F16, tag=f"a16_{gi}")
                for t in range(KT):
                    ft = fp.tile([P, gsz, HW], F16, tag=f"fm{t}")
                    nc.sync.dma_start(ft, fm_d[t, :, g0:g0 + gsz, :])
                    if do_reduce:
                        with nc.allow_low_precision("DVE accumulates fp32"):
                            nc.vector.tensor_reduce(att16[:, t, :], ft,
                                                    axis=AXIS_X, op=ADD)
                if not do_reduce:
                    nc.vector.memset(att16[:, 0, :gsz - gsz % 2].bitcast(F32), 0.0)
                atts[oi] = att16
                # W[e] on the Act HWDGE queue, dispatched BEFORE the previous
                # group's evicts so the Act sequencer never head-of-line
                # blocks the weight stream behind PE-dependent work.
                # The final group's W streams as 3 sub-tiles so its matmuls
                # trail the stream and the post-DMA tail chain is minimal.
                weng = nc.sync if all_sync else nc.scalar
                if tail_split and oi == n - 1:
                    wt = []
                    for j, (a0, aw) in enumerate(WSPLITS):
                        wtj = wtp.tile([P, KT, aw], F16, tag=f"wtl{j}")
                        weng.dma_start(wtj, wt_d[e][:, :, a0:a0 + aw])
                        wt.append(wtj)
                else:
                    wt = wtp.tile([P, KT, A], F16, tag="wt")
                    weng.dma_start(wt, wt_d[e])
                wts[oi] = wt

            pending = {}

            def gemm_stage(oi):
                gi = order[oi]
                g0, gsz, e = groups[gi]
                att16, wt = atts.pop(oi), wts.pop(oi)
                ot = op.tile([P, A], F16, tag="out")
                for (c0, cw) in CHUNKS:
                    if not do_mm:
                        continue
                    if isinstance(wt, list):
                        j = next(i for i, (a0, aw) in enumerate(WSPLITS)
                                 if a0 <= c0 < a0 + aw)
                        wtj, cj = wt[j], c0 - WSPLITS[j][0]
                    else:
                        wtj, cj = wt, c0
                    ps = pmm.tile([P, 512], F32, name="ps")
                    for t in range(KT):
                        nc.tensor.matmul(
                            ps[:gsz, :cw],
                            lhsT=att16[:, t, :],
                            rhs=wtj[:, t, cj:cj + cw],
                            start=(t == 0), stop=(t == KT - 1))
                    nc.scalar.copy(ot[:gsz, c0:c0 + cw], ps[:gsz, :cw])
                if not do_mm:
                    nc.vector.memset(ot[:gsz, :2].bitcast(F32), 0.0)
                pending[oi] = (g0, gsz, ot)

            def out_stage(oi):
                # emitted a group late so the out dma's wait (on evicts)
                # never head-of-line blocks the fm/W dispatch FIFO
                g0, gsz, ot = pending.pop(oi)
                if do_out:
                    # SWDGE (gpsimd) traffic degrades aggregate HBM BW
                    # (bwbench: 353 -> 278 GB/s); keep outputs on the HWDGEs.
                    eng = nc.sync if (all_sync or oi % 2 == 0) else nc.scalar
                    if tail_split and oi == n - 1:
                        # final write in halves: the first half leaves while
                        # the last chunks still evict
                        h = (len(CHUNKS) // 2) * 512
                        eng.dma_start(part_d[g0:g0 + gsz, :h], ot[:gsz, :h])
                        eng.dma_start(part_d[g0:g0 + gsz, h:], ot[:gsz, h:])
                    else:
                        eng.dma_start(part_d[g0:g0 + gsz, :], ot[:gsz])

            skew, olag = 1, 2
            for i in range(n + skew + olag):
                if i < n:
                    load_stage(i)
                if skew <= i < n + skew:
                    gemm_stage(i - skew)
                if i >= skew + olag:
                    out_stage(i - skew - olag)

    nc.compile()
    return nc


_PROGRAM_CACHE = {}


def _get_program(groups):
    key = tuple(groups)
    if key not in _PROGRAM_CACHE:
        _PROGRAM_CACHE[key] = build_program(groups)
    return _PROGRAM_CACHE[key]


def make_in_maps(mask, features, W, b, inst):
    """Host-side routing + sharding.  Returns (in_maps, perm, groups)."""
    inst_np = np.asarray(inst).astype(np.int64)
    perm = np.argsort(inst_np, kind="stable")
    counts = np.bincount(inst_np, minlength=E)
    groups = _make_groups(counts)

    m = np.asarray(mask, np.float64).reshape(B, HW) + 1e-10
    mn = (m / m.sum(1, keepdims=True)).astype(np.float32)[perm]

    feat = np.asarray(features, np.float32).reshape(B, C, HW)[perm]
    # fold the mask into the fp16 cast: fm[s, c, hw] = feat * mn
    fm16 = (feat * mn[:, None, :]).astype(np.float16)
    Wf = np.asarray(W, np.float32)

    in_maps = []
    for k in range(NCORES):
        sl = slice(k * CS, (k + 1) * CS)
        # fm_k[t, p, s, hw] = fm[s, k*CS + t*128 + p, hw]
        fm_k = np.ascontiguousarray(
            fm16[:, sl].reshape(B, KT, P, HW).transpose(1, 2, 0, 3))
        # wt_k[e, p, t, a] = W[e, a, k*CS + t*128 + p]
        wt_k = np.ascontiguousarray(
            Wf[:, :, sl].transpose(0, 2, 1).reshape(E, KT, P, A)
            .transpose(0, 2, 1, 3)).astype(np.float16)
        in_maps.append({
            "fm": fm_k,
            "wt": wt_k,
        })
    return in_maps, perm, groups


def postprocess(results, perm, b, inst):
    part = np.zeros((B, A), np.float32)
    for r in results:
        part += np.asarray(r["part"], np.float32)
    out = np.empty((B, A), np.float32)
    out[perm] = part
    out += np.asarray(b, np.float32)[np.asarray(inst).astype(np.int64)]
    return out


def kernel(mask, features, W, b, inst):
    in_maps, perm, groups = make_in_maps(mask, features, W, b, inst)
    nc = _get_program(groups)
    res = bass_utils.run_bass_kernel_spmd(nc, in_maps, core_ids=list(range(NCORES)))
    return postprocess(res.results, perm, b, inst)


# revision 34
# speedup vs baseline: 1.0213x; 1.0213x over previous
"""
MoE-routing kernel for Trainium2 (8 NeuronCores, SPMD via bass).

Computation (matches the reference):
  attended[b, c] = sum_hw(mn[b, hw] * feat[b, c, hw]),  mn = (m+1e-10)/sum(m+1e-10)
  out[b, a]      = attended[b, :] @ W[inst[b], a, :] + bias[inst[b], a]

Strategy: channel-sharded over 8 cores (CS = 2048/8 = 256 channels each);
host sums the 8 partial [B, A] outputs and adds the bias.  Samples are
sorted by expert on the host so each expert's samples form a contiguous
range.  The mask multiply is folded into the host-side fp16 cast of feat
(fm = feat * mn), so on device the pooling is a pure free-axis sum that
runs on the DVE — the PE does only the expert GEMM.

The kernel is HBM-bandwidth bound: ~52 MB/core (fm 25.7 + W 24.6 + out
1.5 MB) against a measured ~353 GB/s per-core ceiling (shared across all
queues; SWDGE participation degrades it, so everything rides the two
HWDGE queues).  Per expert group (<=128 samples of one expert), emitted
largest-first and software-pipelined with a one-group skew:
  load stage:  per-kt fm tiles [128c, gsz, 196hw] stream on the SP
     HWDGE queue; DVE tensor_reduce sums hw directly into fp16
     att16 [128, KT, gsz] (internal fp32 accumulate); W[e] [128, KT, A]
     is dispatched on the Act HWDGE queue BEFORE the previous group's
     evictions so the weight stream never waits on PE-dependent work.
  gemm stage (one group behind): per 512-answer chunk, KT fp16 matmuls
     accumulate in PSUM; Act evicts to an SBUF row tile.
  out stage (two groups behind): the row tile DMAs to DRAM on an HWDGE
     queue, lagged so its eviction-dependent wait never head-of-line
     blocks fm/W dispatch in the engine FIFOs.
"""

import sys

if "/opt/trn_rl_repo" not in sys.path:
    sys.path.insert(0, "/opt/trn_rl_repo")

import numpy as np

import concourse.bass as bass
import concourse.mybir as mybir
import concourse.tile as tile
from concourse import bacc
from concourse import bass_utils

# Problem constants (hardcoded; kernel.py must be self-contained)
B = 256          # batch
C = 2048         # channels
HW = 196         # spatial positions (14*14)
E = 16           # experts
A = 3000         # answers
NCORES = 8
CS = C // NCORES  # channel shard per core = 256
P = 128
KT = CS // P      # channel k-tiles per core = 2
CHUNKS = [(c0, min(512, A - c0)) for c0 in range(0, A, 512)]
# W sub-tiles per group: matmuls on early answers start while later answers
# still stream, shortening the end-of-iteration tail.
WSPLITS = [(0, 1024), (1024, 1024), (2048, 952)]

F32 = mybir.dt.float32
F16 = mybir.dt.float16
AXIS_X = mybir.AxisListType.X
ADD = mybir.AluOpType.add


def _make_groups(counts):
    """[(gstart_in_sorted_order, gsz, expert)] with gsz <= 128."""
    groups = []
    start = 0
    for e in range(E):
        n = int(counts[e])
        g0 = start
        while n > 0:
            gsz = min(n, P)
            groups.append((g0, gsz, e))
            g0 += gsz
            n -= gsz
        start += int(counts[e])
    return groups


def build_program(groups, loop_n=1, do_reduce=True, do_mm=True, do_out=True,
                  fm_bufs=4, wsplit=False, reorder=True, fm_fused=False,
                  all_sync=False, tail_split=False, skew=1, wt_bufs=3,
                  ps_bufs=4, olag=2, outs_on="alt"):
    """Build + compile the per-core Bass program (identical on all cores).

    do_reduce/do_mm/do_out strip compute stages for sim ablations; the
    correctness path always uses the defaults."""
    nc = bacc.Bacc("TRN2", target_bir_lowering=False, debug=False,
                   num_devices=NCORES)

    fm_d = nc.dram_tensor("fm", [KT, P, B, HW], F16, kind="ExternalInput").ap()
    wt_d = nc.dram_tensor("wt", [E, P, KT, A], F16, kind="ExternalInput").ap()
    part_d = nc.dram_tensor("part", [B, A], F16, kind="ExternalOutput").ap()

    import contextlib
    with tile.TileContext(nc) as tc:
        loop_ctx = tc.For_i(0, loop_n, 1) if loop_n > 1 else contextlib.nullcontext()
        with (
            loop_ctx,
            tc.tile_pool(name="persist", bufs=1) as pp,
            tc.tile_pool(name="fm", bufs=fm_bufs) as fp,
            tc.tile_pool(name="wt", bufs=wt_bufs) as wtp,
            tc.tile_pool(name="outs", bufs=2) as op,
            tc.tile_pool(name="ps_mm", bufs=ps_bufs, space="PSUM") as pmm,
        ):
            # emit largest groups first so the end-of-iteration tail chain
            # runs on the smallest group
            order = (sorted(range(len(groups)), key=lambda i: -groups[i][1])
                     if reorder else list(range(len(groups))))
            n = len(order)
            atts, wts = {}, {}

            def load_stage(oi):
                gi = order[oi]
                g0, gsz, e = groups[gi]
                # pooling: fm -> att16 (per-kt DMA + DVE reduce)
                att16 = pp.tile([P, KT, gsz], F16, tag=f"a16_{gi}")
                for t in range(KT):
                    ft = fp.tile([P, gsz, HW], F16, tag=f"fm{t}")
                    nc.sync.dma_start(ft, fm_d[t, :, g0:g0 + gsz, :])
                    if do_reduce:
                        with nc.allow_low_precision("DVE accumulates fp32"):
                            nc.vector.tensor_reduce(att16[:, t, :], ft,
                                                    axis=AXIS_X, op=ADD)
                if not do_reduce:
                    nc.vector.memset(att16[:, 0, :gsz - gsz % 2].bitcast(F32), 0.0)
                atts[oi] = att16
                # W[e] on the Act HWDGE queue, dispatched BEFORE the previous
                # group's evicts so the Act sequencer never head-of-line
                # blocks the weight stream behind PE-dependent work.
                # The final group's W streams as 3 sub-tiles so its matmuls
                # trail the stream and the post-DMA tail chain is minimal.
                weng = nc.sync if all_sync else nc.scalar
                if tail_split and oi == n - 1:
                    wt = []
                    for j, (a0, aw) in enumerate(WSPLITS):
                        wtj = wtp.tile([P, KT, aw], F16, tag=f"wtl{j}")
                        weng.dma_start(wtj, wt_d[e][:, :, a0:a0 + aw])
                        wt.append(wtj)
                else:
                    wt = wtp.tile([P, KT, A], F16, tag="wt")
                    weng.dma_start(wt, wt_d[e])
                wts[oi] = wt

            pending = {}

            def gemm_stage(oi):
                gi = order[oi]
                g0, gsz, e = groups[gi]
                att16, wt = atts.pop(oi), wts.pop(oi)
                ot = op.tile([P, A], F16, tag="out")
                for (c0, cw) in CHUNKS:
                    if not do_mm:
                        continue
                    if isinstance(wt, list):
                        j = next(i for i, (a0, aw) in enumerate(WSPLITS)
                                 if a0 <= c0 < a0 + aw)
                        wtj, cj = wt[j], c0 - WSPLITS[j][0]
                    else:
                        wtj, cj = wt, c0
                    ps = pmm.tile([P, 512], F32, name="ps")
                    for t in range(KT):
                        nc.tensor.matmul(
                            ps[:gsz, :cw],
                            lhsT=att16[:, t, :],
                            rhs=wtj[:, t, cj:cj + cw],
                            start=(t == 0), stop=(t == KT - 1))
                    nc.scalar.copy(ot[:gsz, c0:c0 + cw], ps[:gsz, :cw])
                if not do_mm:
                    nc.vector.memset(ot[:gsz, :2].bitcast(F32), 0.0)
                pending[oi] = (g0, gsz, ot)

            def out_stage(oi):
                # emitted a group late so the out dma's wait (on evicts)
                # never head-of-line blocks the fm/W dispatch FIFO
                g0, gsz, ot = pending.pop(oi)
                if do_out:
                    # SWDGE (gpsimd) traffic degrades aggregate HBM BW
                    # (bwbench: 353 -> 278 GB/s); keep outputs on the HWDGEs.
                    if outs_on == "sync":
                        eng = nc.sync
                    elif outs_on == "scalar":
                        eng = nc.scalar
                    else:
                        eng = nc.sync if (all_sync or oi % 2 == 0) else nc.scalar
                    if tail_split and oi == n - 1:
                        # final write in halves: the first half leaves while
                        # the last chunks still evict
                        h = (len(CHUNKS) // 2) * 512
                        eng.dma_start(part_d[g0:g0 + gsz, :h], ot[:gsz, :h])
                        eng.dma_start(part_d[g0:g0 + gsz, h:], ot[:gsz, h:])
                    else:
                        eng.dma_start(part_d[g0:g0 + gsz, :], ot[:gsz])

            for i in range(n + skew + olag):
                if i < n:
                    load_stage(i)
                if skew <= i < n + skew:
                    gemm_stage(i - skew)
                if i >= skew + olag:
                    out_stage(i - skew - olag)

    nc.compile()
    return nc


_PROGRAM_CACHE = {}


def _get_program(groups):
    key = tuple(groups)
    if key not in _PROGRAM_CACHE:
        _PROGRAM_CACHE[key] = build_program(groups)
    return _PROGRAM_CACHE[key]


def make_in_maps(mask, features, W, b, inst):
    """Host-side routing + sharding.  Returns (in_maps, perm, groups)."""
    inst_np = np.asarray(inst).astype(np.int64)
    perm = np.argsort(inst_np, kind="stable")
    counts = np.bincount(inst_np, minlength=E)
    groups = _make_groups(counts)

    m = np.asarray(mask, np.float64).reshape(B, HW) + 1e-10
    mn = (m / m.sum(1, keepdims=True)).astype(np.float32)[perm]

    feat = np.asarray(features, np.float32).reshape(B, C, HW)[perm]
    # fold the mask into the fp16 cast: fm[s, c, hw] = feat * mn
    fm16 = (feat * mn[:, None, :]).astype(np.float16)
    Wf = np.asarray(W, np.float32)

    in_maps = []
    for k in range(NCORES):
        sl = slice(k * CS, (k + 1) * CS)
        # fm_k[t, p, s, hw] = fm[s, k*CS + t*128 + p, hw]
        fm_k = np.ascontiguousarray(
            fm16[:, sl].reshape(B, KT, P, HW).transpose(1, 2, 0, 3))
        # wt_k[e, p, t, a] = W[e, a, k*CS + t*128 + p]
        wt_k = np.ascontiguousarray(
            Wf[:, :, sl].transpose(0, 2, 1).reshape(E, KT, P, A)
            .transpose(0, 2, 1, 3)).astype(np.float16)
        in_maps.append({
            "fm": fm_k,
            "wt": wt_k,
        })
    return in_maps, perm, groups


def postprocess(results, perm, b, inst):
    part = np.zeros((B, A), np.float32)
    for r in results:
        part += np.asarray(r["part"], np.float32)
    out = np.empty((B, A), np.float32)
    out[perm] = part
    out += np.asarray(b, np.float32)[np.asarray(inst).astype(np.int64)]
    return out


def kernel(mask, features, W, b, inst):
    in_maps, perm, groups = make_in_maps(mask, features, W, b, inst)
    nc = _get_program(groups)
    res = bass_utils.run_bass_kernel_spmd(nc, in_maps, core_ids=list(range(NCORES)))
    return postprocess(res.results, perm, b, inst)
